# revision 31
# baseline (speedup 1.0000x reference)
"""HGRN2Block kernel for 8 TRN2 NeuronCores.

Live path of the reference (the recurrence is dead code):
    x_proj = x @ W_proj + b_proj            # [B,L,3D]
    gate, _, ogate = split(x_proj, 3)       # middle third is DEAD
    out = gate * sigmoid(ogate)             # [B,L,D]
    out = out @ W_out + b_out               # [B,L,D]

Strategy (v3):
  - Data-parallel over B*L rows: 16384 rows -> 2048 rows/core, no collectives.
  - o-projection in fp8 e4m3 DoubleRow (2 k-slices per matmul, 4 matmuls per
    group): rel_err ~1.5e-2 vs the 2e-2 budget (sigmoid' damps the noise).
    o-weights pre-scaled x16 into e4m3's normal range; sigmoid descales.
  - GLOBAL phases (o-all -> h-all -> L2-all over all 4 row blocks) instead
    of per-row-block phases: the o-all boot needs only fp8 data (small), all
    later phases have >5us of DMA slack, and there are 2 phase boundaries
    instead of 23. All 32 sig/g tiles stay resident (~170KB/partition).
  - ONE PSUM tag, bufs=8: every phase cycles all 8 banks, so a group-start
    waits for a consumer 7 groups back (~6us slack) — no consumer-lag stalls.
  - Boot: fused fp8 tensor pk0 = [w_m0 | x8_k0-3 | w_m1 | x8_k4-7 | w_m2,m3]
    split into 3 fat-line slice DMAs, so the first DR matmul is gated by
    0.375MB (~10us) instead of 1MB (~12.6us). 6 PE warm-up spins bridge the
    HAM clock-gate (~3.4us of activity to ungate) during the DMA prologue.
  - Single sync-ring DMA order = need order; small-line DMAs avoided (early
    ring throughput collapses below ~2KB per-partition lines). y stores:
    halves for rb0-2, quarters for rb3 (short tail).
"""

import os

import numpy as np
import ml_dtypes

try:
    import concourse.bass as bass
except ImportError:
    import sys

    sys.path.insert(0, "/opt/trn_rl_repo")
    import concourse.bass as bass

import concourse.mybir as mybir
from concourse import bacc
from concourse.tile import TileContext
from concourse.bass_utils import run_bass_kernel_spmd

BF16 = ml_dtypes.bfloat16
F8E4 = ml_dtypes.float8_e4m3  # TRN fp8_e4m3 variant (max +-240)

B, L, D = 4, 4096, 1024
NCORES = 8
ROWS = B * L            # 16384
RPC = ROWS // NCORES    # 2048 rows per core
RB = 512                # moving free-dim per matmul (= one fp32 PSUM bank)
NRB = RPC // RB         # 4 row blocks per core
P = 128                 # SBUF partitions
KT = D // P             # 8 contraction tiles
WSCALE = 16.0           # o-proj weight pre-scale (descaled in the sigmoid)

_NC = None
LAST_RESULT = None      # BassKernelResults of the most recent run (for test.py)


def _build():
    nc = bacc.Bacc(trn_type="TRN2")
    f32 = mybir.dt.float32
    bf16 = mybir.dt.bfloat16
    f8 = mybir.dt.float8e4
    DR = mybir.MatmulPerfMode.DoubleRow

    # Boot tensor, 16 cols of 512 fp8 (1MB, 3-4KB lines):
    #   cols 0:2   = wo8 chunks 0:8    (m0 weights)
    #   cols 2:6   = x8 rb0 k=0..3
    #   cols 6:8   = wo8 chunks 8:16   (m1 weights)
    #   cols 8:12  = x8 rb0 k=4..7
    #   cols 12:16 = wo8 chunks 16:32  (m2, m3 weights)
    pk0 = nc.dram_tensor("pk0", [P, 2 * KT, RB], f8, kind="ExternalInput")
    # fp8 x for rbs 1-3, flat [p, (rb-1)*8+k, cb]
    x8 = nc.dram_tensor("x8", [P, (NRB - 1) * KT, RB], f8, kind="ExternalInput")
    xb = nc.dram_tensor("xb", [NRB, P, KT, RB], bf16, kind="ExternalInput")
    # o-weights second half (chunks 32:64 = m4..m7)
    wo8 = nc.dram_tensor("wo8", [P, KT * KT // 2, P], f8, kind="ExternalInput")
    wg = nc.dram_tensor("wg", [P, KT * KT, P], bf16, kind="ExternalInput")
    wu = nc.dram_tensor("wu", [P, KT * KT, P], bf16, kind="ExternalInput")
    # all three biases in one [128, 24] tensor: columns [bg | bo | bu]
    bb = nc.dram_tensor("bb", [P, 3 * KT], f32, kind="ExternalInput")
    # y row-block-wide: [rb, p, n*512+cb] = out[rb*512+cb, n*128+p]
    y = nc.dram_tensor("y", [NRB, P, KT * RB], bf16, kind="ExternalOutput")

    with TileContext(nc) as tc:
        with (
            tc.tile_pool(name="const", bufs=1) as cpool,
            tc.tile_pool(name="outp", bufs=2) as opool,
            tc.tile_pool(name="ps", bufs=8, space="PSUM") as pspool,
        ):
            bbS = cpool.tile([P, 3 * KT], f32, tag="bb", name="bbS")
            bgS = bbS[:, 0:KT]
            boS = bbS[:, KT:2 * KT]
            buS = bbS[:, 2 * KT:3 * KT]

            # HAM warm-up spins (no DMA deps) during the DMA prologue.
            wz = cpool.tile([P, RB], bf16, tag="wz", name="wz")
            nc.vector.memset(wz, 0.0)
            for i in range(9):
                spin = pspool.tile([P, RB], f32, tag="ps", name=f"spin{i}")
                nc.tensor.matmul(spin, lhsT=wz[:, :P], rhs=wz, start=True, stop=True)

            pk0S = cpool.tile([P, 2 * KT, RB], f8, tag="pk0", name="pk0S")
            x8S = cpool.tile([P, (NRB - 1) * KT, RB], f8, tag="x8", name="x8S")
            xbS = [cpool.tile([P, KT, RB], bf16, tag=f"xb_{r}", name=f"xbS{r}")
                   for r in range(NRB)]
            wo8S = cpool.tile([P, KT * KT // 2, P], f8, tag="wo8", name="wo8S")
            wgS = cpool.tile([P, KT * KT, P], bf16, tag="wg", name="wgS")
            wuS = cpool.tile([P, KT * KT, P], bf16, tag="wu", name="wuS")

            # Single fast (sync/SP) HWDGE ring, exact need order.
            def dsync(dst, src):
                nc.sync.dma_start(out=dst, in_=src)

            dsync(pk0S[:, 0:6, :], pk0[:, 0:6, :])      # m0 w + x8 k0-3
            dsync(pk0S[:, 6:12, :], pk0[:, 6:12, :])    # m1 w + x8 k4-7
            dsync(pk0S[:, 12:16, :], pk0[:, 12:16, :])  # m2, m3 w
            dsync(wo8S, wo8[:, :, :])                   # m4-7 w
            for r in range(NRB - 1):                    # fp8 x rb1-3
                dsync(x8S[:, r * KT:(r + 1) * KT, :], x8[:, r * KT:(r + 1) * KT, :])
            dsync(xbS[0], xb[0, :, :, :])
            dsync(xbS[1], xb[1, :, :, :])
            W4 = KT * KT // 4
            for q in range(4):
                dsync(wgS[:, q * W4:(q + 1) * W4, :], wg[:, q * W4:(q + 1) * W4, :])
            dsync(xbS[2], xb[2, :, :, :])
            dsync(xbS[3], xb[3, :, :, :])
            W2 = KT * KT // 2
            for h in range(2):
                dsync(wuS[:, h * W2:(h + 1) * W2, :], wu[:, h * W2:(h + 1) * W2, :])

            # scalar HWDGE ring: only the small bias tile.
            nc.scalar.dma_start(out=bbS, in_=bb[:, :])

            def o_weight(s):
                # DR lhsT [128, 2, 128] for w chunks s, s+1 (s even)
                if s < 32:
                    if s < 8:
                        c = s // 4
                    elif s < 16:
                        c = 6 + (s - 8) // 4
                    else:
                        c = 12 + (s - 16) // 4
                    lw = pk0S[:, c, (s % 4) * P:(s % 4) * P + 2 * P]
                    return lw.rearrange("p (two m) -> p two m", two=2)
                return wo8S[:, s - 32:s - 32 + 2, :]

            def o_rhs(rb, j):
                # DR rhs [128, 2, 512] for x8 chunks 2j, 2j+1 of row block rb
                if rb == 0:
                    c = 2 + 2 * j if j < 2 else 8 + 2 * (j - 2)
                    return pk0S[:, c:c + 2, :]
                return x8S[:, (rb - 1) * KT + 2 * j:(rb - 1) * KT + 2 * j + 2, :]

            # ---- o-all: fp8 DoubleRow, 4 matmuls per group, 32 groups ----
            sigs = [[None] * KT for _ in range(NRB)]

            def o_mm(po, rb, m, j):
                nc.tensor.matmul(
                    po, lhsT=o_weight(m * KT + 2 * j), rhs=o_rhs(rb, j),
                    start=(j == 0), stop=(j == KT // 2 - 1),
                    perf_mode=DR,
                )

            def o_sig(po, rb, m):
                sig = cpool.tile([P, RB], bf16, tag=f"sig{rb}_{m}",
                                 name=f"sig{rb}_{m}")
                nc.scalar.activation(
                    out=sig, in_=po,
                    func=mybir.ActivationFunctionType.Sigmoid,
                    bias=boS[:, m:m + 1], scale=1.0 / WSCALE,
                )
                sigs[rb][m] = sig

            for rb in range(NRB):
                for m in range(KT):
                    po = pspool.tile([P, RB], f32, tag="ps", name=f"po{rb}_{m}")
                    for j in range(KT // 2):
                        o_mm(po, rb, m, j)
                    o_sig(po, rb, m)
            # ---- h-all: bf16 gate proj; g = (h + bg) * sig, 32 groups ----
            gS = [[None] * KT for _ in range(NRB)]
            for rb in range(NRB):
                for m in range(KT):
                    ph = pspool.tile([P, RB], f32, tag="ps", name=f"ph{rb}_{m}")
                    for k in range(KT):
                        s = m * KT + k
                        nc.tensor.matmul(
                            ph, lhsT=wgS[:, s:s + 1, :],
                            rhs=xbS[rb][:, k:k + 1, :],
                            start=(k == 0), stop=(k == KT - 1),
                        )
                    g = cpool.tile([P, RB], bf16, tag=f"g{rb}_{m}",
                                   name=f"g{rb}_{m}")
                    nc.vector.scalar_tensor_tensor(
                        out=g, in0=ph, scalar=bgS[:, m:m + 1], in1=sigs[rb][m],
                        op0=mybir.AluOpType.add, op1=mybir.AluOpType.mult,
                    )
                    gS[rb][m] = g
            # ---- L2-all: y = g @ W_out (+ b_out), bf16, rb-wide out ----
            for rb in range(NRB):
                yo = opool.tile([P, KT * RB], bf16, tag="yo",
                                name=f"yo{rb}", bufs=2)
                for n in range(KT):
                    py = pspool.tile([P, RB], f32, tag="ps", name=f"py{rb}_{n}")
                    for m in range(KT):
                        s = n * KT + m
                        nc.tensor.matmul(
                            py, lhsT=wuS[:, s:s + 1, :], rhs=gS[rb][m],
                            start=(m == 0), stop=(m == KT - 1),
                        )
                    # bias-add on ScalarE (idle during layer 2)
                    nc.scalar.activation(
                        out=yo[:, n * RB:(n + 1) * RB], in_=py,
                        func=mybir.ActivationFunctionType.Identity,
                        bias=buS[:, n:n + 1], scale=1.0,
                    )
                    if rb < NRB - 1:
                        # halves: few descriptors, 4KB lines
                        if n == KT // 2 - 1:
                            nc.sync.dma_start(out=y[rb, :, 0:KT * RB // 2],
                                              in_=yo[:, 0:KT * RB // 2])
                        elif n == KT - 1:
                            nc.sync.dma_start(out=y[rb, :, KT * RB // 2:KT * RB],
                                              in_=yo[:, KT * RB // 2:KT * RB])
                    elif n < KT - 1:
                        # last rb: per-n eighths for the shortest tail
                        nc.sync.dma_start(
                            out=y[rb, :, n * RB:(n + 1) * RB],
                            in_=yo[:, n * RB:(n + 1) * RB])
                    else:
                        # final store on the scalar queue: same engine as its
                        # identity, so no cross-engine semaphore hop
                        nc.scalar.dma_start(
                            out=y[rb, :, n * RB:(n + 1) * RB],
                            in_=yo[:, n * RB:(n + 1) * RB])
    nc.finalize()
    return nc


def kernel(x, W_proj, b_proj, W_out, b_out, layer_idx=0, num_layers=12):
    global _NC, LAST_RESULT
    x = np.asarray(x, dtype=np.float32)
    W_proj = np.asarray(W_proj, dtype=np.float32)
    b_proj = np.asarray(b_proj, dtype=np.float32)
    W_out = np.asarray(W_out, dtype=np.float32)
    b_out = np.asarray(b_out, dtype=np.float32)

    Wg = W_proj[:, :D]
    Wo = W_proj[:, 2 * D:3 * D]

    def pack_w(w):
        # [D, D] -> [p, a*8+b, c] with out[p, a*8+b, c] = w[b*128+p, a*128+c]
        return np.ascontiguousarray(
            w.reshape(KT, P, KT, P).transpose(1, 2, 0, 3).reshape(P, KT * KT, P)
        )

    wgp = pack_w(Wg).astype(BF16)
    wo8p = pack_w(Wo * WSCALE).astype(F8E4)   # [P, 64, P] chunks m*8+k
    wup = pack_w(W_out).astype(BF16)
    bbp = np.ascontiguousarray(np.concatenate([
        b_proj[:D].reshape(KT, P).T,
        b_proj[2 * D:3 * D].reshape(KT, P).T,
        b_out.reshape(KT, P).T,
    ], axis=1))
    # w chunks as flat 512-cols: wflat[:, c, :] = chunks 4c:4c+4
    wflat = wo8p.reshape(P, KT * KT // 4, 4 * P)

    xf = x.reshape(ROWS, D)
    in_maps = []
    for c in range(NCORES):
        # [rb, p, k, cb] with xc[rb, p, k, cb] = x_core[rb*512+cb, k*128+p]
        xc = np.ascontiguousarray(
            xf[c * RPC:(c + 1) * RPC].reshape(NRB, RB, KT, P).transpose(0, 3, 2, 1)
        )
        xc8 = xc.astype(F8E4)
        # boot: [w_m0 | x8_k0-3 | w_m1 | x8_k4-7 | w_m2,m3]
        pk0c = np.ascontiguousarray(np.concatenate([
            wflat[:, 0:2, :].reshape(P, 2, RB),
            xc8[0, :, 0:4, :],
            wflat[:, 2:4, :].reshape(P, 2, RB),
            xc8[0, :, 4:8, :],
            wflat[:, 4:8, :].reshape(P, 4, RB),
        ], axis=1))
        x8c = np.ascontiguousarray(
            xc8[1:].transpose(1, 0, 2, 3).reshape(P, (NRB - 1) * KT, RB))
        in_maps.append({
            "pk0": pk0c, "x8": x8c, "xb": xc.astype(BF16),
            "wo8": np.ascontiguousarray(wo8p[:, 32:64, :]),
            "wg": wgp, "wu": wup, "bb": bbp,
        })

    if _NC is None:
        _NC = _build()

    trace = os.environ.get("HGRN_TRACE", "0") == "1"
    LAST_RESULT = run_bass_kernel_spmd(
        _NC, in_maps, core_ids=list(range(NCORES)), trace=trace,
        tmpdir=os.environ.get("HGRN_TMPDIR"),
    )
    yout = np.empty((ROWS, D), dtype=np.float32)
    for c in range(NCORES):
        yc = np.asarray(LAST_RESULT.results[c]["y"])  # [rb, p, 4096] bf16
        yc = yc.reshape(NRB, P, KT, RB)               # [rb, p, n, cb]
        yout[c * RPC:(c + 1) * RPC] = (
            yc.transpose(0, 3, 2, 1).reshape(RPC, D).astype(np.float32)
        )
    return yout.reshape(B, L, D)


# revision 32
# speedup vs baseline: 1.0072x; 1.0072x over previous
"""HGRN2Block kernel for 8 TRN2 NeuronCores.

Live path of the reference (the recurrence is dead code):
    x_proj = x @ W_proj + b_proj            # [B,L,3D]
    gate, _, ogate = split(x_proj, 3)       # middle third is DEAD
    out = gate * sigmoid(ogate)             # [B,L,D]
    out = out @ W_out + b_out               # [B,L,D]

Strategy (v3):
  - Data-parallel over B*L rows: 16384 rows -> 2048 rows/core, no collectives.
  - o-projection in fp8 e4m3 DoubleRow (2 k-slices per matmul, 4 matmuls per
    group): rel_err ~1.5e-2 vs the 2e-2 budget (sigmoid' damps the noise).
    o-weights pre-scaled x16 into e4m3's normal range; sigmoid descales.
  - GLOBAL phases (o-all -> h-all -> L2-all over all 4 row blocks) instead
    of per-row-block phases: the o-all boot needs only fp8 data (small), all
    later phases have >5us of DMA slack, and there are 2 phase boundaries
    instead of 23. All 32 sig/g tiles stay resident (~170KB/partition).
  - ONE PSUM tag, bufs=8: every phase cycles all 8 banks, so a group-start
    waits for a consumer 7 groups back (~6us slack) — no consumer-lag stalls.
  - Boot: fused fp8 tensor pk0 = [w_m0 | x8_k0-3 | w_m1 | x8_k4-7 | w_m2,m3]
    split into 3 fat-line slice DMAs, so the first DR matmul is gated by
    0.375MB (~10us) instead of 1MB (~12.6us). 6 PE warm-up spins bridge the
    HAM clock-gate (~3.4us of activity to ungate) during the DMA prologue.
  - Single sync-ring DMA order = need order; small-line DMAs avoided (early
    ring throughput collapses below ~2KB per-partition lines). y stores:
    halves for rb0-2, quarters for rb3 (short tail).
"""

import os

import numpy as np
import ml_dtypes

try:
    import concourse.bass as bass
except ImportError:
    import sys

    sys.path.insert(0, "/opt/trn_rl_repo")
    import concourse.bass as bass

import concourse.mybir as mybir
from concourse import bacc
from concourse.tile import TileContext
from concourse.bass_utils import run_bass_kernel_spmd

BF16 = ml_dtypes.bfloat16
F8E4 = ml_dtypes.float8_e4m3  # TRN fp8_e4m3 variant (max +-240)

B, L, D = 4, 4096, 1024
NCORES = 8
ROWS = B * L            # 16384
RPC = ROWS // NCORES    # 2048 rows per core
RB = 512                # moving free-dim per matmul (= one fp32 PSUM bank)
NRB = RPC // RB         # 4 row blocks per core
P = 128                 # SBUF partitions
KT = D // P             # 8 contraction tiles
WSCALE = 16.0           # o-proj weight pre-scale (descaled in the sigmoid)

_NC = None
LAST_RESULT = None      # BassKernelResults of the most recent run (for test.py)


def _build():
    nc = bacc.Bacc(trn_type="TRN2")
    f32 = mybir.dt.float32
    bf16 = mybir.dt.bfloat16
    f8 = mybir.dt.float8e4
    DR = mybir.MatmulPerfMode.DoubleRow

    # Boot tensor, 16 cols of 512 fp8 (1MB, 3-4KB lines):
    #   cols 0:2   = wo8 chunks 0:8    (m0 weights)
    #   cols 2:6   = x8 rb0 k=0..3
    #   cols 6:8   = wo8 chunks 8:16   (m1 weights)
    #   cols 8:12  = x8 rb0 k=4..7
    #   cols 12:16 = wo8 chunks 16:32  (m2, m3 weights)
    pk0 = nc.dram_tensor("pk0", [P, 2 * KT, RB], f8, kind="ExternalInput")
    # fp8 x for rbs 1-3, flat [p, (rb-1)*8+k, cb]
    x8 = nc.dram_tensor("x8", [P, (NRB - 1) * KT, RB], f8, kind="ExternalInput")
    xb = nc.dram_tensor("xb", [NRB, P, KT, RB], bf16, kind="ExternalInput")
    # o-weights second half (chunks 32:64 = m4..m7)
    wo8 = nc.dram_tensor("wo8", [P, KT * KT // 2, P], f8, kind="ExternalInput")
    wg = nc.dram_tensor("wg", [P, KT * KT, P], bf16, kind="ExternalInput")
    wu = nc.dram_tensor("wu", [P, KT * KT, P], bf16, kind="ExternalInput")
    # all three biases in one [128, 24] tensor: columns [bg | bo | bu]
    bb = nc.dram_tensor("bb", [P, 3 * KT], f32, kind="ExternalInput")
    # y row-block-wide: [rb, p, n*512+cb] = out[rb*512+cb, n*128+p]
    y = nc.dram_tensor("y", [NRB, P, KT * RB], bf16, kind="ExternalOutput")

    with TileContext(nc) as tc:
        with (
            tc.tile_pool(name="const", bufs=1) as cpool,
            tc.tile_pool(name="outp", bufs=2) as opool,
            tc.tile_pool(name="ps", bufs=8, space="PSUM") as pspool,
        ):
            bbS = cpool.tile([P, 3 * KT], f32, tag="bb", name="bbS")
            bgS = bbS[:, 0:KT]
            boS = bbS[:, KT:2 * KT]
            buS = bbS[:, 2 * KT:3 * KT]

            # HAM warm-up spins (no DMA deps) during the DMA prologue.
            wz = cpool.tile([P, RB], bf16, tag="wz", name="wz")
            nc.vector.memset(wz, 0.0)
            for i in range(8):
                spin = pspool.tile([P, RB], f32, tag="ps", name=f"spin{i}")
                nc.tensor.matmul(spin, lhsT=wz[:, :P], rhs=wz, start=True, stop=True)

            pk0S = cpool.tile([P, 2 * KT, RB], f8, tag="pk0", name="pk0S")
            x8S = cpool.tile([P, (NRB - 1) * KT, RB], f8, tag="x8", name="x8S")
            xbS = [cpool.tile([P, KT, RB], bf16, tag=f"xb_{r}", name=f"xbS{r}")
                   for r in range(NRB)]
            wo8S = cpool.tile([P, KT * KT // 2, P], f8, tag="wo8", name="wo8S")
            wgS = cpool.tile([P, KT * KT, P], bf16, tag="wg", name="wgS")
            wuS = cpool.tile([P, KT * KT, P], bf16, tag="wu", name="wuS")

            # Single fast (sync/SP) HWDGE ring, exact need order.
            def dsync(dst, src):
                nc.sync.dma_start(out=dst, in_=src)

            dsync(pk0S[:, 0:6, :], pk0[:, 0:6, :])      # m0 w + x8 k0-3
            dsync(pk0S[:, 6:12, :], pk0[:, 6:12, :])    # m1 w + x8 k4-7
            dsync(pk0S[:, 12:16, :], pk0[:, 12:16, :])  # m2, m3 w
            dsync(wo8S, wo8[:, :, :])                   # m4-7 w
            for r in range(NRB - 1):                    # fp8 x rb1-3
                dsync(x8S[:, r * KT:(r + 1) * KT, :], x8[:, r * KT:(r + 1) * KT, :])
            dsync(xbS[0], xb[0, :, :, :])
            dsync(xbS[1], xb[1, :, :, :])
            W4 = KT * KT // 4
            for q in range(4):
                dsync(wgS[:, q * W4:(q + 1) * W4, :], wg[:, q * W4:(q + 1) * W4, :])
            dsync(xbS[2], xb[2, :, :, :])
            dsync(xbS[3], xb[3, :, :, :])
            W2 = KT * KT // 2
            for h in range(2):
                dsync(wuS[:, h * W2:(h + 1) * W2, :], wu[:, h * W2:(h + 1) * W2, :])

            # scalar HWDGE ring: only the small bias tile.
            nc.scalar.dma_start(out=bbS, in_=bb[:, :])

            def o_weight(s):
                # DR lhsT [128, 2, 128] for w chunks s, s+1 (s even)
                if s < 32:
                    if s < 8:
                        c = s // 4
                    elif s < 16:
                        c = 6 + (s - 8) // 4
                    else:
                        c = 12 + (s - 16) // 4
                    lw = pk0S[:, c, (s % 4) * P:(s % 4) * P + 2 * P]
                    return lw.rearrange("p (two m) -> p two m", two=2)
                return wo8S[:, s - 32:s - 32 + 2, :]

            def o_rhs(rb, j):
                # DR rhs [128, 2, 512] for x8 chunks 2j, 2j+1 of row block rb
                if rb == 0:
                    c = 2 + 2 * j if j < 2 else 8 + 2 * (j - 2)
                    return pk0S[:, c:c + 2, :]
                return x8S[:, (rb - 1) * KT + 2 * j:(rb - 1) * KT + 2 * j + 2, :]

            # ---- o-all: fp8 DoubleRow, 4 matmuls per group, 32 groups ----
            sigs = [[None] * KT for _ in range(NRB)]

            def o_mm(po, rb, m, j):
                nc.tensor.matmul(
                    po, lhsT=o_weight(m * KT + 2 * j), rhs=o_rhs(rb, j),
                    start=(j == 0), stop=(j == KT // 2 - 1),
                    perf_mode=DR,
                )

            def o_sig(po, rb, m):
                sig = cpool.tile([P, RB], bf16, tag=f"sig{rb}_{m}",
                                 name=f"sig{rb}_{m}")
                nc.scalar.activation(
                    out=sig, in_=po,
                    func=mybir.ActivationFunctionType.Sigmoid,
                    bias=boS[:, m:m + 1], scale=1.0 / WSCALE,
                )
                sigs[rb][m] = sig

            for rb in range(NRB):
                for m in range(KT):
                    po = pspool.tile([P, RB], f32, tag="ps", name=f"po{rb}_{m}")
                    for j in range(KT // 2):
                        o_mm(po, rb, m, j)
                    o_sig(po, rb, m)
            # ---- h-all: bf16 gate proj; g = (h + bg) * sig, 32 groups ----
            gS = [[None] * KT for _ in range(NRB)]
            for rb in range(NRB):
                for m in range(KT):
                    ph = pspool.tile([P, RB], f32, tag="ps", name=f"ph{rb}_{m}")
                    for k in range(KT):
                        s = m * KT + k
                        nc.tensor.matmul(
                            ph, lhsT=wgS[:, s:s + 1, :],
                            rhs=xbS[rb][:, k:k + 1, :],
                            start=(k == 0), stop=(k == KT - 1),
                        )
                    g = cpool.tile([P, RB], bf16, tag=f"g{rb}_{m}",
                                   name=f"g{rb}_{m}")
                    nc.vector.scalar_tensor_tensor(
                        out=g, in0=ph, scalar=bgS[:, m:m + 1], in1=sigs[rb][m],
                        op0=mybir.AluOpType.add, op1=mybir.AluOpType.mult,
                    )
                    gS[rb][m] = g
            # ---- L2-all: y = g @ W_out (+ b_out), bf16, rb-wide out ----
            for rb in range(NRB):
                yo = opool.tile([P, KT * RB], bf16, tag="yo",
                                name=f"yo{rb}", bufs=2)
                for n in range(KT):
                    py = pspool.tile([P, RB], f32, tag="ps", name=f"py{rb}_{n}")
                    for m in range(KT):
                        s = n * KT + m
                        nc.tensor.matmul(
                            py, lhsT=wuS[:, s:s + 1, :], rhs=gS[rb][m],
                            start=(m == 0), stop=(m == KT - 1),
                        )
                    # bias-add on ScalarE (idle during layer 2)
                    nc.scalar.activation(
                        out=yo[:, n * RB:(n + 1) * RB], in_=py,
                        func=mybir.ActivationFunctionType.Identity,
                        bias=buS[:, n:n + 1], scale=1.0,
                    )
                    if rb < NRB - 1:
                        # halves: few descriptors, 4KB lines
                        if n == KT // 2 - 1:
                            nc.sync.dma_start(out=y[rb, :, 0:KT * RB // 2],
                                              in_=yo[:, 0:KT * RB // 2])
                        elif n == KT - 1:
                            nc.sync.dma_start(out=y[rb, :, KT * RB // 2:KT * RB],
                                              in_=yo[:, KT * RB // 2:KT * RB])
                    elif n < KT - 1:
                        # last rb: per-n eighths for the shortest tail
                        nc.sync.dma_start(
                            out=y[rb, :, n * RB:(n + 1) * RB],
                            in_=yo[:, n * RB:(n + 1) * RB])
                    else:
                        # final store on the scalar queue: same engine as its
                        # identity, so no cross-engine semaphore hop
                        nc.scalar.dma_start(
                            out=y[rb, :, n * RB:(n + 1) * RB],
                            in_=yo[:, n * RB:(n + 1) * RB])
    nc.finalize()
    return nc


def kernel(x, W_proj, b_proj, W_out, b_out, layer_idx=0, num_layers=12):
    global _NC, LAST_RESULT
    x = np.asarray(x, dtype=np.float32)
    W_proj = np.asarray(W_proj, dtype=np.float32)
    b_proj = np.asarray(b_proj, dtype=np.float32)
    W_out = np.asarray(W_out, dtype=np.float32)
    b_out = np.asarray(b_out, dtype=np.float32)

    Wg = W_proj[:, :D]
    Wo = W_proj[:, 2 * D:3 * D]

    def pack_w(w):
        # [D, D] -> [p, a*8+b, c] with out[p, a*8+b, c] = w[b*128+p, a*128+c]
        return np.ascontiguousarray(
            w.reshape(KT, P, KT, P).transpose(1, 2, 0, 3).reshape(P, KT * KT, P)
        )

    wgp = pack_w(Wg).astype(BF16)
    wo8p = pack_w(Wo * WSCALE).astype(F8E4)   # [P, 64, P] chunks m*8+k
    wup = pack_w(W_out).astype(BF16)
    bbp = np.ascontiguousarray(np.concatenate([
        b_proj[:D].reshape(KT, P).T,
        b_proj[2 * D:3 * D].reshape(KT, P).T,
        b_out.reshape(KT, P).T,
    ], axis=1))
    # w chunks as flat 512-cols: wflat[:, c, :] = chunks 4c:4c+4
    wflat = wo8p.reshape(P, KT * KT // 4, 4 * P)

    xf = x.reshape(ROWS, D)
    in_maps = []
    for c in range(NCORES):
        # [rb, p, k, cb] with xc[rb, p, k, cb] = x_core[rb*512+cb, k*128+p]
        xc = np.ascontiguousarray(
            xf[c * RPC:(c + 1) * RPC].reshape(NRB, RB, KT, P).transpose(0, 3, 2, 1)
        )
        xc8 = xc.astype(F8E4)
        # boot: [w_m0 | x8_k0-3 | w_m1 | x8_k4-7 | w_m2,m3]
        pk0c = np.ascontiguousarray(np.concatenate([
            wflat[:, 0:2, :].reshape(P, 2, RB),
            xc8[0, :, 0:4, :],
            wflat[:, 2:4, :].reshape(P, 2, RB),
            xc8[0, :, 4:8, :],
            wflat[:, 4:8, :].reshape(P, 4, RB),
        ], axis=1))
        x8c = np.ascontiguousarray(
            xc8[1:].transpose(1, 0, 2, 3).reshape(P, (NRB - 1) * KT, RB))
        in_maps.append({
            "pk0": pk0c, "x8": x8c, "xb": xc.astype(BF16),
            "wo8": np.ascontiguousarray(wo8p[:, 32:64, :]),
            "wg": wgp, "wu": wup, "bb": bbp,
        })

    if _NC is None:
        _NC = _build()

    trace = os.environ.get("HGRN_TRACE", "0") == "1"
    LAST_RESULT = run_bass_kernel_spmd(
        _NC, in_maps, core_ids=list(range(NCORES)), trace=trace,
        tmpdir=os.environ.get("HGRN_TMPDIR"),
    )
    yout = np.empty((ROWS, D), dtype=np.float32)
    for c in range(NCORES):
        yc = np.asarray(LAST_RESULT.results[c]["y"])  # [rb, p, 4096] bf16
        yc = yc.reshape(NRB, P, KT, RB)               # [rb, p, n, cb]
        yout[c * RPC:(c + 1) * RPC] = (
            yc.transpose(0, 3, 2, 1).reshape(RPC, D).astype(np.float32)
        )
    return yout.reshape(B, L, D)
